# revision 1
# baseline (speedup 1.0000x reference)
"""Trainium2 Bass kernel for CommutatorConv2d.

Math: with lambda_c=0, lambda_a=1 the reference is a conv2d with effective
kernel  w_eff[o,i,r,s] = krow[o,i,s] + kcol[o,i,r]  (krow = sum_r w, kcol =
sum_s w), plus bias.  That kernel lives in a 6-dim matrix subspace, so the
9-tap conv factors into two 1D convs over box-summed inputs:

  y[o,h,w] = sum_{i,s} krow[o,i,s] * xv[i, h, w+s-1]
           + sum_{i,r} kcol[o,i,r] * xh[i, h+r-1, w]  + bias[o]

where xv = vertical 3-tap sum of zero-padded x, xh = horizontal 3-tap sum.
Per output tile that is 6 accumulating matmuls (contraction 128 each)
instead of 9 — 2/3 of the PE work of direct conv.

Sharding: data-parallel over batch; 4 images per core on 8 cores.
"""

import os
import numpy as np
import ml_dtypes

import concourse.bass as bass
import concourse.bacc as bacc
import concourse.mybir as mybir
import concourse.tile as tile
from concourse.bass_utils import run_bass_kernel_spmd

B, CI, CO, H, W = 32, 128, 256, 56, 56
NCORES = 8
BPC = B // NCORES          # images per core
HP, WP = H + 2, W + 2      # padded spatial dims
NPIX = H * W               # 3136
ROWT = 8                   # output rows per matmul tile
NT = H // ROWT             # 7 pixel tiles per image
NTILE = ROWT * W           # 448 columns per matmul

F32 = mybir.dt.float32
BF16 = mybir.dt.bfloat16


def build_nc():
    nc = bacc.Bacc(None, enable_partition_id=False)
    xin = nc.declare_dram_parameter("xp", [BPC, CI, HP, WP], BF16, isOutput=False)
    wk = nc.declare_dram_parameter("klhs", [CI, 6, CO], BF16, isOutput=False)
    bb = nc.declare_dram_parameter("bias2", [CI, 2], F32, isOutput=False)
    y = nc.declare_dram_parameter("y", [BPC, CO, H, W], F32, isOutput=True)

    xflat = xin.rearrange("b c h w -> b c (h w)")
    yflat = y.rearrange("b o h w -> b o (h w)")
    NPAD = HP * WP           # 3364
    NV = H * WP              # 3248 (rows 0..55 of padded, all 58 cols)
    NH = NPAD - 2            # 3362 (all padded rows, cols shifted 0/1/2)

    with tile.TileContext(nc) as tc:
        with (
            tc.tile_pool(name="const", bufs=1) as cpool,
            tc.tile_pool(name="xp", bufs=2) as xpool,
            tc.tile_pool(name="xv", bufs=2) as vpool,
            tc.tile_pool(name="xh", bufs=2) as hpool,
            tc.tile_pool(name="yo", bufs=4) as ypool,
            tc.tile_pool(name="ps", bufs=7, space="PSUM") as pspool,
        ):
            klhs_sb = cpool.tile([CI, 6 * CO], BF16)
            bias_sb = cpool.tile([CI, 2], F32)
            kl3 = klhs_sb.rearrange("i (t o) -> i t o", o=CO)

            # PE warmup: a burst of tiny matmuls on zeros, issued while the
            # first input DMAs are in flight, trips the HAM clock-gate to
            # 2.4 GHz before the real matmul stream begins (saves ~4us of
            # half-rate ramp).
            warm = cpool.tile([128, NTILE], BF16)
            nc.gpsimd.memset(warm[:], 0.0)
            warm_ps = pspool.tile([128, NTILE], F32, bufs=1, tag="warm")
            for _ in range(7):
                nc.tensor.matmul(
                    warm_ps[:], warm[:, 0:128], warm[:], start=True, stop=True
                )
            warm_out = cpool.tile([128, 32], F32)
            nc.scalar.activation(
                warm_out[:], warm_ps[:, 0:32], mybir.ActivationFunctionType.Copy
            )

            for b in range(BPC):
                # Image 0 gates the whole pipeline: load it in row-chunks so
                # box-sums (and then matmuls) start as soon as possible.
                row_chunks = [12, 32, HP] if b == 0 else [HP]

                xp_sb = xpool.tile([CI, NPAD], BF16)
                xp3d = xflat[b].rearrange("i (h w) -> i h w", w=WP)
                xps3 = xp_sb.rearrange("i (h w) -> i h w", w=WP)
                r0 = 0
                for ci, r1 in enumerate(row_chunks):
                    nc.sync.dma_start(out=xps3[:, r0:r1, :], in_=xp3d[:, r0:r1, :])
                    if b == 0 and ci == 0:
                        # weights/bias queued right after the first chunk
                        nc.sync.dma_start(
                            out=klhs_sb[:], in_=wk.rearrange("i t o -> i (t o)")
                        )
                        nc.sync.dma_start(out=bias_sb[:], in_=bb[:])
                    r0 = r1

                # box-sums, emitted per DMA chunk so they overlap the loads:
                # xv[j] = xp[j] + xp[j+58] + xp[j+116]   (rows 0..55)
                # xh[j] = xp[j] + xp[j+1] + xp[j+2]      (rows 0..57, garbage
                #                                         at cols 56/57 unused)
                xvt = vpool.tile([CI, NV], BF16)
                xv = vpool.tile([CI, NV], BF16)
                xht = hpool.tile([CI, NPAD], BF16)
                xh = hpool.tile([CI, NPAD], BF16)
                heng = nc.vector
                v0 = h0r = 0
                for ci, r1 in enumerate(row_chunks):
                    last = ci == len(row_chunks) - 1
                    v1 = H if last else r1 - 2        # xv rows ready
                    h1 = HP if last else r1 - 1       # xh rows ready
                    a, z = v0 * WP, v1 * WP
                    nc.vector.tensor_add(
                        xvt[:, a:z], xp_sb[:, a:z], xp_sb[:, a + WP : z + WP]
                    )
                    nc.vector.tensor_add(
                        xv[:, a:z], xvt[:, a:z], xp_sb[:, a + 2 * WP : z + 2 * WP]
                    )
                    a, z = h0r * WP, h1 * WP - 2
                    heng.tensor_add(
                        xht[:, a:z], xp_sb[:, a:z], xp_sb[:, a + 1 : z + 1]
                    )
                    heng.tensor_add(
                        xh[:, a:z], xht[:, a:z], xp_sb[:, a + 2 : z + 2]
                    )
                    v0, h0r = v1, h1

                xv3 = xv.rearrange("i (h w) -> i h w", w=WP)   # [128, 56, 58]
                xh3 = xh.rearrange("i (h w) -> i h w", w=WP)   # [128, 58, 58]

                youts = {}

                def emit(half, t, b=b, xv3=xv3, xh3=xh3, youts=youts):
                    if half not in youts:
                        youts[half] = ypool.tile(
                            [128, NPIX], F32, name=f"yout_{b}_{half}", tag="yout"
                        )
                    yout = youts[half]
                    h0 = t * ROWT
                    ps = pspool.tile([128, NTILE], F32, name=f"ps_{b}_{half}_{t}", tag="ps")
                    for s in range(3):
                        nc.tensor.matmul(
                            ps[:],
                            kl3[:, s, half * 128 : half * 128 + 128],
                            xv3[:, h0 : h0 + ROWT, s : s + W],
                            start=(s == 0),
                            stop=False,
                        )
                    for r in range(3):
                        nc.tensor.matmul(
                            ps[:],
                            kl3[:, 3 + r, half * 128 : half * 128 + 128],
                            xh3[:, h0 + r : h0 + r + ROWT, 0:W],
                            start=False,
                            stop=(r == 2),
                        )
                    nc.scalar.activation(
                        yout[:, t * NTILE : (t + 1) * NTILE],
                        ps[:],
                        mybir.ActivationFunctionType.Identity,
                        bias=bias_sb[:, half : half + 1],
                    )
                    last_block = b == BPC - 1 and half == 1
                    if t == 3:
                        nc.sync.dma_start(
                            out=yflat[b, half * 128 : half * 128 + 128, 0 : 4 * NTILE],
                            in_=yout[:, 0 : 4 * NTILE],
                        )
                    elif t >= 4 and last_block:
                        # final block: per-tile stores so the kernel tail
                        # only waits on one small DMA
                        nc.sync.dma_start(
                            out=yflat[
                                b,
                                half * 128 : half * 128 + 128,
                                t * NTILE : (t + 1) * NTILE,
                            ],
                            in_=yout[:, t * NTILE : (t + 1) * NTILE],
                        )
                    if t == NT - 1 and not last_block:
                        nc.sync.dma_start(
                            out=yflat[b, half * 128 : half * 128 + 128, 4 * NTILE : NPIX],
                            in_=yout[:, 4 * NTILE : NPIX],
                        )

                if b == 0:
                    # image 0: both halves' A-chunk tiles first, so the PE
                    # never stalls on the late B-segment box-sums
                    order = (
                        [(0, t) for t in range(3)]
                        + [(1, t) for t in range(3)]
                        + [(0, t) for t in range(3, NT)]
                        + [(1, t) for t in range(3, NT)]
                    )
                else:
                    order = [(h, t) for h in range(2) for t in range(NT)]
                for half, t in order:
                    emit(half, t)
    nc.finalize()
    return nc


_NC_CACHE = {}


def _get_nc():
    if "nc" not in _NC_CACHE:
        _NC_CACHE["nc"] = build_nc()
    return _NC_CACHE["nc"]


def make_in_maps(x, weight, bias):
    x = np.asarray(x, dtype=np.float32)
    weight = np.asarray(weight, dtype=np.float32)
    bias = np.asarray(bias, dtype=np.float32)

    krow = weight.sum(axis=3)  # [O, I, 3]
    kcol = weight.sum(axis=2)  # [O, I, 3]
    klhs = np.empty((CI, 6, CO), np.float32)
    for s in range(3):
        klhs[:, s, :] = krow[:, :, s].T
        klhs[:, 3 + s, :] = kcol[:, :, s].T
    klhs = klhs.astype(ml_dtypes.bfloat16)

    xp = np.zeros((B, CI, HP, WP), np.float32)
    xp[:, :, 1 : H + 1, 1 : W + 1] = x
    xp = xp.astype(ml_dtypes.bfloat16)

    bias2 = np.ascontiguousarray(bias.reshape(2, 128).T)  # [128, 2] f32

    return [
        {"xp": xp[c * BPC : (c + 1) * BPC], "klhs": klhs, "bias2": bias2}
        for c in range(NCORES)
    ]


def run(in_maps, **kwargs):
    nc = _get_nc()
    return run_bass_kernel_spmd(nc, in_maps, list(range(NCORES)), **kwargs)


def kernel(x, weight, bias):
    res = run(make_in_maps(x, weight, bias))
    return np.concatenate([res.results[c]["y"] for c in range(NCORES)], axis=0)



# revision 2
# speedup vs baseline: 1.1226x; 1.1226x over previous
"""Trainium2 Bass kernel for CommutatorConv2d.

Math: with lambda_c=0, lambda_a=1 the reference is a conv2d with effective
kernel  w_eff[o,i,r,s] = krow[o,i,s] + kcol[o,i,r]  (krow = sum_r w, kcol =
sum_s w).  The 9-tap conv factors into 1D convs over box-summed inputs, and
because sum_s xv_s == sum_r xh_r == P (the 3x3 patch sum), the 6 natural
contraction slices collapse to 5:

  y[o,h,w] = (krow0-krow2)[o,i] * xv[i, h, w]
           + (krow1-krow2)[o,i] * xv[i, h, w+1]
           + (kcol0-kcol2)[o,i] * xh[i, h,   w]
           + (kcol1-kcol2)[o,i] * xh[i, h+1, w]
           + (krow2+kcol2)[o,i] * P [i, h, w]      + bias[o]

where xv = vertical 3-tap sum of zero-padded x, xh = horizontal 3-tap sum,
P = horizontal 3-tap sum of xv.  Per output tile that is 5 accumulating
matmuls (contraction 128 each) instead of 9 for direct conv / 6 for the
two-1D-conv form.  Weight differences are computed host-side for free.

Output is stored bf16 on device (halves store DMA) and upcast on host.

Sharding: data-parallel over batch; 4 images per core on 8 cores.
"""

import numpy as np
import ml_dtypes

import concourse.bass as bass
import concourse.bacc as bacc
import concourse.mybir as mybir
import concourse.tile as tile
from concourse.bass_utils import run_bass_kernel_spmd

B, CI, CO, H, W = 32, 128, 256, 56, 56
NCORES = 8
BPC = B // NCORES          # images per core
HP, WP = H + 2, W + 2      # padded spatial dims
NPIX = H * W               # 3136
ROWT = 8                   # output rows per matmul tile
NT = H // ROWT             # 7 pixel tiles per image
NTILE = ROWT * W           # 448 columns per matmul
NSL = 5                    # contraction slices

F32 = mybir.dt.float32
BF16 = mybir.dt.bfloat16


def build_nc():
    nc = bacc.Bacc(None, enable_partition_id=False)
    xin = nc.declare_dram_parameter("xp", [BPC, CI, HP, WP], BF16, isOutput=False)
    wk = nc.declare_dram_parameter("klhs", [CI, NSL, CO], BF16, isOutput=False)
    bb = nc.declare_dram_parameter("bias2", [CI, 2], F32, isOutput=False)
    y = nc.declare_dram_parameter("y", [BPC, CO, H, W], BF16, isOutput=True)

    xflat = xin.rearrange("b c h w -> b c (h w)")
    yflat = y.rearrange("b o h w -> b o (h w)")
    NPAD = HP * WP           # 3364
    NV = H * WP              # 3248 (rows 0..55 of padded, all 58 cols)

    with tile.TileContext(nc) as tc:
        with (
            tc.tile_pool(name="const", bufs=1) as cpool,
            tc.tile_pool(name="xp", bufs=2) as xpool,
            tc.tile_pool(name="xv", bufs=2) as vpool,
            tc.tile_pool(name="xh", bufs=2) as hpool,
            tc.tile_pool(name="pp", bufs=2) as ppool,
            tc.tile_pool(name="yo", bufs=4) as ypool,
            tc.tile_pool(name="ps", bufs=7, space="PSUM") as pspool,
        ):
            klhs_sb = cpool.tile([CI, NSL * CO], BF16)
            bias_sb = cpool.tile([CI, 2], F32)
            kl3 = klhs_sb.rearrange("i (t o) -> i t o", o=CO)

            # PE warmup: a burst of tiny matmuls on zeros, issued while the
            # first input DMAs are in flight, trips the HAM clock-gate to
            # 2.4 GHz before the real matmul stream begins.
            warm = cpool.tile([128, NTILE], BF16)
            nc.gpsimd.memset(warm[:], 0.0)
            warm_ps = pspool.tile([128, NTILE], F32, bufs=1, tag="warm")
            for _ in range(7):
                nc.tensor.matmul(
                    warm_ps[:], warm[:, 0:128], warm[:], start=True, stop=True
                )
            warm_out = cpool.tile([128, 32], F32)
            nc.scalar.activation(
                warm_out[:], warm_ps[:, 0:32], mybir.ActivationFunctionType.Copy
            )

            for b in range(BPC):
                # Image 0 gates the whole pipeline: load it in row-chunks so
                # box-sums (and then matmuls) start as soon as possible.
                row_chunks = [12, 32, HP] if b == 0 else [HP]

                xp_sb = xpool.tile([CI, NPAD], BF16)
                xp3d = xflat[b].rearrange("i (h w) -> i h w", w=WP)
                xps3 = xp_sb.rearrange("i (h w) -> i h w", w=WP)
                r0 = 0
                for ci, r1 in enumerate(row_chunks):
                    nc.sync.dma_start(out=xps3[:, r0:r1, :], in_=xp3d[:, r0:r1, :])
                    if b == 0 and ci == 0:
                        # weights/bias queued right after the first chunk
                        nc.sync.dma_start(
                            out=klhs_sb[:], in_=wk.rearrange("i t o -> i (t o)")
                        )
                        nc.sync.dma_start(out=bias_sb[:], in_=bb[:])
                    r0 = r1

                # box-sums, emitted per DMA chunk so they overlap the loads:
                # xv[j] = xp[j] + xp[j+58] + xp[j+116]   (rows 0..55)
                # xh[j] = xp[j] + xp[j+1] + xp[j+2]      (rows 0..57, garbage
                #                                         at cols 56/57 unused)
                # P[j]  = xv[j] + xv[j+1] + xv[j+2]      (rows 0..55, ditto)
                xvt = vpool.tile([CI, NV], BF16)
                xv = vpool.tile([CI, NV], BF16)
                xht = hpool.tile([CI, NPAD], BF16)
                xh = hpool.tile([CI, NPAD], BF16)
                ppt = ppool.tile([CI, NV], BF16)
                pp = ppool.tile([CI, NV], BF16)
                v0 = h0r = 0
                for ci, r1 in enumerate(row_chunks):
                    last = ci == len(row_chunks) - 1
                    v1 = H if last else r1 - 2        # xv rows ready
                    h1 = HP if last else r1 - 1       # xh rows ready
                    a, z = v0 * WP, v1 * WP
                    nc.vector.tensor_add(
                        xvt[:, a:z], xp_sb[:, a:z], xp_sb[:, a + WP : z + WP]
                    )
                    nc.vector.tensor_add(
                        xv[:, a:z], xvt[:, a:z], xp_sb[:, a + 2 * WP : z + 2 * WP]
                    )
                    a, z = h0r * WP, h1 * WP - 2
                    nc.vector.tensor_add(
                        xht[:, a:z], xp_sb[:, a:z], xp_sb[:, a + 1 : z + 1]
                    )
                    nc.vector.tensor_add(
                        xh[:, a:z], xht[:, a:z], xp_sb[:, a + 2 : z + 2]
                    )
                    a, z = v0 * WP, v1 * WP - 2
                    nc.vector.tensor_add(
                        ppt[:, a:z], xv[:, a:z], xv[:, a + 1 : z + 1]
                    )
                    nc.vector.tensor_add(
                        pp[:, a:z], ppt[:, a:z], xv[:, a + 2 : z + 2]
                    )
                    v0, h0r = v1, h1

                xv3 = xv.rearrange("i (h w) -> i h w", w=WP)   # [128, 56, 58]
                xh3 = xh.rearrange("i (h w) -> i h w", w=WP)   # [128, 58, 58]
                pp3 = pp.rearrange("i (h w) -> i h w", w=WP)   # [128, 56, 58]

                youts = {}
                pstiles = {}

                def rhs_for(s, t, xv3=xv3, xh3=xh3, pp3=pp3):
                    h0 = t * ROWT
                    if s == 0:
                        return xv3[:, h0 : h0 + ROWT, 0:W]
                    if s == 1:
                        return xv3[:, h0 : h0 + ROWT, 1 : 1 + W]
                    if s == 2:
                        return xh3[:, h0 : h0 + ROWT, 0:W]
                    if s == 3:
                        return xh3[:, h0 + 1 : h0 + 1 + ROWT, 0:W]
                    return pp3[:, h0 : h0 + ROWT, 0:W]

                def emit(half, s, t, b=b, youts=youts, pstiles=pstiles):
                    if half not in youts:
                        youts[half] = ypool.tile(
                            [128, NPIX], BF16, name=f"yout_{b}_{half}", tag="yout"
                        )
                    yout = youts[half]
                    if (half, t) not in pstiles:
                        pstiles[(half, t)] = pspool.tile(
                            [128, NTILE], F32, name=f"ps_{b}_{half}_{t}", tag="ps"
                        )
                    ps = pstiles[(half, t)]
                    nc.tensor.matmul(
                        ps[:],
                        kl3[:, s, half * 128 : half * 128 + 128],
                        rhs_for(s, t),
                        start=(s == 0),
                        stop=(s == NSL - 1),
                    )
                    if s != NSL - 1:
                        return
                    # last slice: drain PSUM -> SBUF (bf16, +bias), then DMA
                    nc.scalar.activation(
                        yout[:, t * NTILE : (t + 1) * NTILE],
                        ps[:],
                        mybir.ActivationFunctionType.Identity,
                        bias=bias_sb[:, half : half + 1],
                    )
                    last_block = b == BPC - 1 and half == 1
                    if t == 3:
                        nc.sync.dma_start(
                            out=yflat[b, half * 128 : half * 128 + 128, 0 : 4 * NTILE],
                            in_=yout[:, 0 : 4 * NTILE],
                        )
                    elif t >= 4 and last_block:
                        # final block: per-tile stores so the kernel tail
                        # only waits on one small DMA
                        nc.sync.dma_start(
                            out=yflat[
                                b,
                                half * 128 : half * 128 + 128,
                                t * NTILE : (t + 1) * NTILE,
                            ],
                            in_=yout[:, t * NTILE : (t + 1) * NTILE],
                        )
                    if t == NT - 1 and not last_block:
                        nc.sync.dma_start(
                            out=yflat[b, half * 128 : half * 128 + 128, 4 * NTILE : NPIX],
                            in_=yout[:, 4 * NTILE : NPIX],
                        )

                # slice-major order: each slice's weights stay stationary
                # across a run of tiles.
                if b == 0:
                    # image 0 arrives in row-chunks; tiles 0-2 are ready
                    # after chunk 2, so run them (both halves, all slices)
                    # before the tail tiles.
                    order = (
                        [(h, s, t) for h in range(2) for s in range(NSL) for t in range(3)]
                        + [(h, s, t) for h in range(2) for s in range(NSL) for t in range(3, NT)]
                    )
                else:
                    order = [
                        (h, s, t)
                        for h in range(2)
                        for s in range(NSL)
                        for t in range(NT)
                    ]
                for half, s, t in order:
                    emit(half, s, t)
    nc.finalize()
    return nc


_NC_CACHE = {}


def _get_nc():
    if "nc" not in _NC_CACHE:
        _NC_CACHE["nc"] = build_nc()
    return _NC_CACHE["nc"]


def make_in_maps(x, weight, bias):
    x = np.asarray(x, dtype=np.float32)
    weight = np.asarray(weight, dtype=np.float32)
    bias = np.asarray(bias, dtype=np.float32)

    krow = weight.sum(axis=3)  # [O, I, 3]
    kcol = weight.sum(axis=2)  # [O, I, 3]
    klhs = np.empty((CI, NSL, CO), np.float32)
    klhs[:, 0, :] = (krow[:, :, 0] - krow[:, :, 2]).T
    klhs[:, 1, :] = (krow[:, :, 1] - krow[:, :, 2]).T
    klhs[:, 2, :] = (kcol[:, :, 0] - kcol[:, :, 2]).T
    klhs[:, 3, :] = (kcol[:, :, 1] - kcol[:, :, 2]).T
    klhs[:, 4, :] = (krow[:, :, 2] + kcol[:, :, 2]).T
    klhs = klhs.astype(ml_dtypes.bfloat16)

    xp = np.zeros((B, CI, HP, WP), np.float32)
    xp[:, :, 1 : H + 1, 1 : W + 1] = x
    xp = xp.astype(ml_dtypes.bfloat16)

    bias2 = np.ascontiguousarray(bias.reshape(2, 128).T)  # [128, 2] f32

    return [
        {"xp": xp[c * BPC : (c + 1) * BPC], "klhs": klhs, "bias2": bias2}
        for c in range(NCORES)
    ]


def run(in_maps, **kwargs):
    nc = _get_nc()
    return run_bass_kernel_spmd(nc, in_maps, list(range(NCORES)), **kwargs)


def kernel(x, weight, bias):
    res = run(make_in_maps(x, weight, bias))
    out = np.concatenate([res.results[c]["y"] for c in range(NCORES)], axis=0)
    return out.astype(np.float32)


# revision 4
# speedup vs baseline: 1.1300x; 1.0066x over previous
"""Trainium2 Bass kernel for CommutatorConv2d.

Math: with lambda_c=0, lambda_a=1 the reference is a conv2d with effective
kernel  w_eff[o,i,r,s] = krow[o,i,s] + kcol[o,i,r]  (krow = sum_r w, kcol =
sum_s w).  The 9-tap conv factors into 1D convs over box-summed inputs, and
because sum_s xv_s == sum_r xh_r == P (the 3x3 patch sum), the 6 natural
contraction slices collapse to 5:

  y[o,h,w] = (krow0-krow2)[o,i] * xv[i, h, w]
           + (krow1-krow2)[o,i] * xv[i, h, w+1]
           + (kcol0-kcol2)[o,i] * xh[i, h,   w]
           + (kcol1-kcol2)[o,i] * xh[i, h+1, w]
           + (krow2+kcol2)[o,i] * P [i, h, w]      + bias[o]

where xv = vertical 3-tap sum of zero-padded x, xh = horizontal 3-tap sum,
P = horizontal 3-tap sum of xv.  Per output tile that is 5 accumulating
matmuls (contraction 128 each) instead of 9 for direct conv / 6 for the
two-1D-conv form.  Weight differences are computed host-side for free.

Schedule: tile-major (a tile's 5 matmuls then its PSUM drain) so drains and
output stores spread across the whole run and the kernel tail is one tile
deep.  Image 0 arrives in 4 row-chunks with box-sums and matmuls emitted
per chunk: the PE stream starts early and never gaps, which also keeps the
HAM clock-gate from re-throttling mid-run.

Output is stored bf16 on device (halves store DMA) and upcast on host.

Sharding: data-parallel over batch; 4 images per core on 8 cores.
"""

import numpy as np
import ml_dtypes

import concourse.bass as bass
import concourse.bacc as bacc
import concourse.mybir as mybir
import concourse.tile as tile
from concourse.bass_utils import run_bass_kernel_spmd

B, CI, CO, H, W = 32, 128, 256, 56, 56
NCORES = 8
BPC = B // NCORES          # images per core
HP, WP = H + 2, W + 2      # padded spatial dims
NPIX = H * W               # 3136
ROWT = 8                   # output rows per matmul tile
NT = H // ROWT             # 7 pixel tiles per image
NTILE = ROWT * W           # 448 columns per matmul
NSL = 5                    # contraction slices

F32 = mybir.dt.float32
BF16 = mybir.dt.bfloat16


def build_nc():
    nc = bacc.Bacc(None, enable_partition_id=False)
    xin = nc.declare_dram_parameter("xp", [BPC, CI, HP, WP], BF16, isOutput=False)
    wk = nc.declare_dram_parameter("klhs", [CI, NSL, CO], BF16, isOutput=False)
    bb = nc.declare_dram_parameter("bias2", [CI, 2], F32, isOutput=False)
    y = nc.declare_dram_parameter("y", [BPC, CO, H, W], BF16, isOutput=True)

    xflat = xin.rearrange("b c h w -> b c (h w)")
    yflat = y.rearrange("b o h w -> b o (h w)")
    NPAD = HP * WP           # 3364
    NV = H * WP              # 3248 (rows 0..55 of padded, all 58 cols)

    with tile.TileContext(nc) as tc:
        with (
            tc.tile_pool(name="const", bufs=1) as cpool,
            tc.tile_pool(name="xp", bufs=2) as xpool,
            tc.tile_pool(name="xv", bufs=2) as vpool,
            tc.tile_pool(name="xh", bufs=2) as hpool,
            tc.tile_pool(name="pp", bufs=2) as ppool,
            tc.tile_pool(name="yo", bufs=4) as ypool,
            tc.tile_pool(name="ps", bufs=7, space="PSUM") as pspool,
        ):
            klhs_sb = cpool.tile([CI, NSL * CO], BF16)
            bias_sb = cpool.tile([CI, 2], F32)
            kl3 = klhs_sb.rearrange("i (t o) -> i t o", o=CO)

            # PE warmup: a burst of tiny matmuls on zeros, issued while the
            # first input DMAs are in flight, trips the HAM clock-gate to
            # 2.4 GHz before the real matmul stream begins.
            warm = cpool.tile([128, NTILE], BF16)
            nc.vector.memset(warm[:], 0.0)
            warm_ps = pspool.tile([128, NTILE], F32, bufs=1, tag="warm")
            for _ in range(7):
                nc.tensor.matmul(
                    warm_ps[:], warm[:, 0:128], warm[:], start=True, stop=True
                )
            warm_out = cpool.tile([128, 32], F32)
            nc.scalar.activation(
                warm_out[:], warm_ps[:, 0:32], mybir.ActivationFunctionType.Copy
            )

            for b in range(BPC):
                # Image 0 gates the whole pipeline: load it in row-chunks so
                # box-sums and matmuls start as soon as possible.  Tiles of
                # image 0 are emitted as soon as their chunk lands, keeping
                # the PE stream gap-free from the first real matmul on.
                if b == 0:
                    row_chunks = [12, 30, 44, HP]
                    tile_groups = [[0], [1, 2], [3, 4], [5, 6]]
                else:
                    row_chunks = [HP]
                    tile_groups = [list(range(NT))]

                xp_sb = xpool.tile([CI, NPAD], BF16)
                xp3d = xflat[b].rearrange("i (h w) -> i h w", w=WP)
                xps3 = xp_sb.rearrange("i (h w) -> i h w", w=WP)

                xvt = vpool.tile([CI, NV], BF16)
                xv = vpool.tile([CI, NV], BF16)
                xht = hpool.tile([CI, NPAD], BF16)
                xh = hpool.tile([CI, NPAD], BF16)
                ppt = ppool.tile([CI, NV], BF16)
                pp = ppool.tile([CI, NV], BF16)

                xv3 = xv.rearrange("i (h w) -> i h w", w=WP)   # [128, 56, 58]
                xh3 = xh.rearrange("i (h w) -> i h w", w=WP)   # [128, 58, 58]
                pp3 = pp.rearrange("i (h w) -> i h w", w=WP)   # [128, 56, 58]

                youts = {}

                def rhs_for(s, t, xv3=xv3, xh3=xh3, pp3=pp3):
                    h0 = t * ROWT
                    if s == 0:
                        return xv3[:, h0 : h0 + ROWT, 0:W]
                    if s == 1:
                        return xv3[:, h0 : h0 + ROWT, 1 : 1 + W]
                    if s == 2:
                        return xh3[:, h0 : h0 + ROWT, 0:W]
                    if s == 3:
                        return xh3[:, h0 + 1 : h0 + 1 + ROWT, 0:W]
                    return pp3[:, h0 : h0 + ROWT, 0:W]

                def emit_tile(half, t, b=b, youts=youts):
                    if half not in youts:
                        youts[half] = ypool.tile(
                            [128, NPIX], BF16, name=f"yout_{b}_{half}", tag="yout"
                        )
                    yout = youts[half]
                    ps = pspool.tile(
                        [128, NTILE], F32, name=f"ps_{b}_{half}_{t}", tag="ps"
                    )
                    for s in range(NSL):
                        nc.tensor.matmul(
                            ps[:],
                            kl3[:, s, half * 128 : half * 128 + 128],
                            rhs_for(s, t),
                            start=(s == 0),
                            stop=(s == NSL - 1),
                        )
                    # drain PSUM -> SBUF (bf16, +bias), then DMA in batches
                    nc.scalar.activation(
                        yout[:, t * NTILE : (t + 1) * NTILE],
                        ps[:],
                        mybir.ActivationFunctionType.Identity,
                        bias=bias_sb[:, half : half + 1],
                    )
                    last_block = b == BPC - 1 and half == 1
                    if t == 3:
                        nc.sync.dma_start(
                            out=yflat[b, half * 128 : half * 128 + 128, 0 : 4 * NTILE],
                            in_=yout[:, 0 : 4 * NTILE],
                        )
                    elif t >= 4 and last_block:
                        # final block: per-tile stores so the kernel tail
                        # only waits on one small DMA
                        nc.sync.dma_start(
                            out=yflat[
                                b,
                                half * 128 : half * 128 + 128,
                                t * NTILE : (t + 1) * NTILE,
                            ],
                            in_=yout[:, t * NTILE : (t + 1) * NTILE],
                        )
                    if t == NT - 1 and not last_block:
                        nc.sync.dma_start(
                            out=yflat[b, half * 128 : half * 128 + 128, 4 * NTILE : NPIX],
                            in_=yout[:, 4 * NTILE : NPIX],
                        )

                if b == 0:
                    # weights/bias first: they gate every matmul, while the
                    # x chunks only gate the (later) box-sums
                    nc.sync.dma_start(
                        out=klhs_sb[:], in_=wk.rearrange("i t o -> i (t o)")
                    )
                    nc.sync.dma_start(out=bias_sb[:], in_=bb[:])

                v0 = h0r = 0
                for ci, r1 in enumerate(row_chunks):
                    c0 = 0 if ci == 0 else row_chunks[ci - 1]
                    nc.sync.dma_start(
                        out=xps3[:, c0:r1, :], in_=xp3d[:, c0:r1, :]
                    )

                    # box-sums for this chunk, then the tiles it unlocks:
                    # xv[j] = xp[j] + xp[j+58] + xp[j+116]   (rows 0..55)
                    # xh[j] = xp[j] + xp[j+1] + xp[j+2]      (rows 0..57)
                    # P[j]  = xv[j] + xv[j+1] + xv[j+2]      (rows 0..55)
                    last = ci == len(row_chunks) - 1
                    v1 = H if last else r1 - 2        # xv rows ready
                    h1 = HP if last else r1 - 1       # xh rows ready
                    a, z = v0 * WP, v1 * WP
                    nc.vector.tensor_add(
                        xvt[:, a:z], xp_sb[:, a:z], xp_sb[:, a + WP : z + WP]
                    )
                    nc.vector.tensor_add(
                        xv[:, a:z], xvt[:, a:z], xp_sb[:, a + 2 * WP : z + 2 * WP]
                    )
                    a, z = h0r * WP, h1 * WP - 2
                    nc.vector.tensor_add(
                        xht[:, a:z], xp_sb[:, a:z], xp_sb[:, a + 1 : z + 1]
                    )
                    nc.vector.tensor_add(
                        xh[:, a:z], xht[:, a:z], xp_sb[:, a + 2 : z + 2]
                    )
                    a, z = v0 * WP, v1 * WP - 2
                    nc.vector.tensor_add(
                        ppt[:, a:z], xv[:, a:z], xv[:, a + 1 : z + 1]
                    )
                    nc.vector.tensor_add(
                        pp[:, a:z], ppt[:, a:z], xv[:, a + 2 : z + 2]
                    )
                    v0, h0r = v1, h1

                    for t in tile_groups[ci]:
                        emit_tile(0, t)
                        emit_tile(1, t)
    nc.finalize()
    return nc


_NC_CACHE = {}


def _get_nc():
    if "nc" not in _NC_CACHE:
        _NC_CACHE["nc"] = build_nc()
    return _NC_CACHE["nc"]


def make_in_maps(x, weight, bias):
    x = np.asarray(x, dtype=np.float32)
    weight = np.asarray(weight, dtype=np.float32)
    bias = np.asarray(bias, dtype=np.float32)

    krow = weight.sum(axis=3)  # [O, I, 3]
    kcol = weight.sum(axis=2)  # [O, I, 3]
    klhs = np.empty((CI, NSL, CO), np.float32)
    klhs[:, 0, :] = (krow[:, :, 0] - krow[:, :, 2]).T
    klhs[:, 1, :] = (krow[:, :, 1] - krow[:, :, 2]).T
    klhs[:, 2, :] = (kcol[:, :, 0] - kcol[:, :, 2]).T
    klhs[:, 3, :] = (kcol[:, :, 1] - kcol[:, :, 2]).T
    klhs[:, 4, :] = (krow[:, :, 2] + kcol[:, :, 2]).T
    klhs = klhs.astype(ml_dtypes.bfloat16)

    xp = np.zeros((B, CI, HP, WP), np.float32)
    xp[:, :, 1 : H + 1, 1 : W + 1] = x
    xp = xp.astype(ml_dtypes.bfloat16)

    bias2 = np.ascontiguousarray(bias.reshape(2, 128).T)  # [128, 2] f32

    return [
        {"xp": xp[c * BPC : (c + 1) * BPC], "klhs": klhs, "bias2": bias2}
        for c in range(NCORES)
    ]


def run(in_maps, **kwargs):
    nc = _get_nc()
    return run_bass_kernel_spmd(nc, in_maps, list(range(NCORES)), **kwargs)


def kernel(x, weight, bias):
    res = run(make_in_maps(x, weight, bias))
    out = np.concatenate([res.results[c]["y"] for c in range(NCORES)], axis=0)
    return out.astype(np.float32)


# revision 10
# speedup vs baseline: 1.1485x; 1.0164x over previous
"""Trainium2 Bass kernel for CommutatorConv2d.

Math: with lambda_c=0, lambda_a=1 the reference is a conv2d with effective
kernel  w_eff[o,i,r,s] = krow[o,i,s] + kcol[o,i,r]  (krow = sum_r w, kcol =
sum_s w).  The 9-tap conv factors into 1D convs over box-summed inputs, and
because sum_s xv_s == sum_r xh_r == P (the 3x3 patch sum), the 6 natural
contraction slices collapse to 5:

  y[o,h,w] = (krow0-krow2)[o,i] * xv[i, h, w]
           + (krow1-krow2)[o,i] * xv[i, h, w+1]
           + (kcol0-kcol2)[o,i] * xh[i, h,   w]
           + (kcol1-kcol2)[o,i] * xh[i, h+1, w]
           + (krow2+kcol2)[o,i] * P [i, h, w]      + bias[o]

where xv = vertical 3-tap sum of zero-padded x, xh = horizontal 3-tap sum,
P = horizontal 3-tap sum of xv.  Per output tile that is 5 accumulating
matmuls (contraction 128 each) instead of 9 for direct conv / 6 for the
two-1D-conv form.  Weight differences are computed host-side for free.

Schedule: tile-major (a tile's 5 matmuls then its PSUM drain) so drains and
output stores spread across the whole run and the kernel tail is one tile
deep.  Image 0 arrives in 4 row-chunks with box-sums and matmuls emitted
per chunk: the PE stream starts early and never gaps, which also keeps the
HAM clock-gate from re-throttling mid-run.

Output is stored bf16 on device (halves store DMA) and upcast on host.

Sharding: data-parallel over batch; 4 images per core on 8 cores.
"""

import numpy as np
import ml_dtypes

import concourse.bass as bass
import concourse.bacc as bacc
import concourse.mybir as mybir
import concourse.tile as tile
from concourse.bass_utils import run_bass_kernel_spmd

B, CI, CO, H, W = 32, 128, 256, 56, 56
NCORES = 8
BPC = B // NCORES          # images per core
HP, WP = H + 2, W + 2      # padded spatial dims
NPIX = H * W               # 3136
ROWT = 8                   # output rows per matmul tile
NT = H // ROWT             # 7 pixel tiles per image
NTILE = ROWT * W           # 448 columns per matmul
NSL = 5                    # contraction slices

F32 = mybir.dt.float32
BF16 = mybir.dt.bfloat16


def build_nc():
    nc = bacc.Bacc(None, enable_partition_id=False)
    xin = nc.declare_dram_parameter("xp", [BPC, CI, HP, WP], BF16, isOutput=False)
    wk = nc.declare_dram_parameter("klhs", [CI, NSL, CO], BF16, isOutput=False)
    bb = nc.declare_dram_parameter("bias2", [CI, 2], F32, isOutput=False)
    y = nc.declare_dram_parameter("y", [BPC, CO, H, W], BF16, isOutput=True)

    xflat = xin.rearrange("b c h w -> b c (h w)")
    yflat = y.rearrange("b o h w -> b o (h w)")
    NPAD = HP * WP           # 3364
    NV = H * WP              # 3248 (rows 0..55 of padded, all 58 cols)

    with tile.TileContext(nc) as tc:
        with (
            tc.tile_pool(name="const", bufs=1) as cpool,
            tc.tile_pool(name="xp", bufs=BPC) as xpool,
            tc.tile_pool(name="xv", bufs=2) as vpool,
            tc.tile_pool(name="xh", bufs=2) as hpool,
            tc.tile_pool(name="pp", bufs=2) as ppool,
            tc.tile_pool(name="yo", bufs=3) as ypool,
            tc.tile_pool(name="ps", bufs=7, space="PSUM") as pspool,
        ):
            klhs_sb = cpool.tile([CI, NSL * CO], BF16)
            bias_sb = cpool.tile([CI, 2], F32)
            kl3 = klhs_sb.rearrange("i (t o) -> i t o", o=CO)

            # PE warmup: a burst of matmuls on zeros bridges the PE from
            # engine-open until the first real matmul's data has landed
            # (DMA + box-sum chain, ~5us), so the HAM clock-gate reaches
            # 2.4 GHz before the real stream begins and never re-throttles.
            warm = cpool.tile([128, NTILE], BF16)
            nc.vector.memset(warm[:], 0.0)
            warm_ps = pspool.tile([128, NTILE], F32, bufs=1, tag="warm")
            for _ in range(11):
                nc.tensor.matmul(
                    warm_ps[:], warm[:, 0:128], warm[:], start=True, stop=True
                )
            warm_out = cpool.tile([128, 32], F32)
            nc.scalar.activation(
                warm_out[:], warm_ps[:, 0:32], mybir.ActivationFunctionType.Copy
            )

            # All input DMAs are issued before any compute/store is emitted:
            # the sync queue issues strictly in program order, so a load
            # emitted after a store would wait for that store's (compute-
            # gated) semaphore — serializing input prefetch behind compute.
            ROW_CHUNKS0 = [12, 30, 44, HP]
            xp_sbs = []
            for b in range(BPC):
                xp_sb = xpool.tile([CI, NPAD], BF16, name=f"xp_{b}")
                xp_sbs.append(xp_sb)
            nc.sync.dma_start(
                out=klhs_sb[:], in_=wk.rearrange("i t o -> i (t o)")
            )
            for b in range(BPC):
                xp3d = xflat[b].rearrange("i (h w) -> i h w", w=WP)
                xps3 = xp_sbs[b].rearrange("i (h w) -> i h w", w=WP)
                chunks = ROW_CHUNKS0 if b == 0 else [HP]
                c0 = 0
                for r1 in chunks:
                    nc.sync.dma_start(out=xps3[:, c0:r1, :], in_=xp3d[:, c0:r1, :])
                    c0 = r1
            nc.sync.dma_start(out=bias_sb[:], in_=bb[:])

            for b in range(BPC):
                # Image 0 gates the whole pipeline: it arrives in row-chunks
                # (DMA'd above) and its box-sums/matmuls are emitted per
                # chunk, so the PE stream starts as soon as chunk 1 lands.
                if b == 0:
                    row_chunks = ROW_CHUNKS0
                    tile_groups = [[0], [1, 2], [3, 4], [5, 6]]
                else:
                    row_chunks = [HP]
                    tile_groups = [list(range(NT))]

                xp_sb = xp_sbs[b]

                # Distinct tags so each tensor gets its own ring: temporaries
                # (read once, immediately) single-buffered; matmul inputs
                # double-buffered so image b+1's box-sums overlap image b's
                # matmuls instead of waiting for its ring slot's last reader.
                xvt = vpool.tile([CI, NV], BF16, tag="xvt", bufs=1)
                xv = vpool.tile([CI, NV], BF16, tag="xv", bufs=2)
                xht = hpool.tile([CI, NPAD], BF16, tag="xht", bufs=1)
                xh = hpool.tile([CI, NPAD], BF16, tag="xh", bufs=2)
                ppt = ppool.tile([CI, NV], BF16, tag="ppt", bufs=1)
                pp = ppool.tile([CI, NV], BF16, tag="pp", bufs=2)

                xv3 = xv.rearrange("i (h w) -> i h w", w=WP)   # [128, 56, 58]
                xh3 = xh.rearrange("i (h w) -> i h w", w=WP)   # [128, 58, 58]
                pp3 = pp.rearrange("i (h w) -> i h w", w=WP)   # [128, 56, 58]

                youts = {}

                def rhs_for(s, t, xv3=xv3, xh3=xh3, pp3=pp3):
                    h0 = t * ROWT
                    if s == 0:
                        return xv3[:, h0 : h0 + ROWT, 0:W]
                    if s == 1:
                        return xv3[:, h0 : h0 + ROWT, 1 : 1 + W]
                    if s == 2:
                        return xh3[:, h0 : h0 + ROWT, 0:W]
                    if s == 3:
                        return xh3[:, h0 + 1 : h0 + 1 + ROWT, 0:W]
                    return pp3[:, h0 : h0 + ROWT, 0:W]

                def emit_tile(half, t, b=b, youts=youts):
                    if half not in youts:
                        youts[half] = ypool.tile(
                            [128, NPIX], BF16, name=f"yout_{b}_{half}", tag="yout"
                        )
                    yout = youts[half]
                    ps = pspool.tile(
                        [128, NTILE], F32, name=f"ps_{b}_{half}_{t}", tag="ps"
                    )
                    for s in range(NSL):
                        nc.tensor.matmul(
                            ps[:],
                            kl3[:, s, half * 128 : half * 128 + 128],
                            rhs_for(s, t),
                            start=(s == 0),
                            stop=(s == NSL - 1),
                        )
                    # drain PSUM -> SBUF (bf16, +bias), then DMA in batches
                    nc.scalar.activation(
                        yout[:, t * NTILE : (t + 1) * NTILE],
                        ps[:],
                        mybir.ActivationFunctionType.Identity,
                        bias=bias_sb[:, half : half + 1],
                    )
                    if b == BPC - 1:
                        # final image: per-tile stores so the kernel tail
                        # only waits on one small DMA
                        nc.sync.dma_start(
                            out=yflat[
                                b,
                                half * 128 : half * 128 + 128,
                                t * NTILE : (t + 1) * NTILE,
                            ],
                            in_=yout[:, t * NTILE : (t + 1) * NTILE],
                        )
                    elif t == 3:
                        nc.sync.dma_start(
                            out=yflat[b, half * 128 : half * 128 + 128, 0 : 4 * NTILE],
                            in_=yout[:, 0 : 4 * NTILE],
                        )
                    elif t == NT - 1:
                        nc.sync.dma_start(
                            out=yflat[b, half * 128 : half * 128 + 128, 4 * NTILE : NPIX],
                            in_=yout[:, 4 * NTILE : NPIX],
                        )

                v0 = h0r = 0
                for ci, r1 in enumerate(row_chunks):

                    # box-sums for this chunk, then the tiles it unlocks:
                    # xv[j] = xp[j] + xp[j+58] + xp[j+116]   (rows 0..55)
                    # xh[j] = xp[j] + xp[j+1] + xp[j+2]      (rows 0..57)
                    # P[j]  = xv[j] + xv[j+1] + xv[j+2]      (rows 0..55)
                    last = ci == len(row_chunks) - 1
                    v1 = H if last else r1 - 2        # xv rows ready
                    h1 = HP if last else r1 - 1       # xh rows ready
                    a, z = v0 * WP, v1 * WP
                    nc.vector.tensor_add(
                        xvt[:, a:z], xp_sb[:, a:z], xp_sb[:, a + WP : z + WP]
                    )
                    nc.vector.tensor_add(
                        xv[:, a:z], xvt[:, a:z], xp_sb[:, a + 2 * WP : z + 2 * WP]
                    )
                    a, z = h0r * WP, h1 * WP - 2
                    nc.vector.tensor_add(
                        xht[:, a:z], xp_sb[:, a:z], xp_sb[:, a + 1 : z + 1]
                    )
                    nc.vector.tensor_add(
                        xh[:, a:z], xht[:, a:z], xp_sb[:, a + 2 : z + 2]
                    )
                    a, z = v0 * WP, v1 * WP - 2
                    nc.vector.tensor_add(
                        ppt[:, a:z], xv[:, a:z], xv[:, a + 1 : z + 1]
                    )
                    nc.vector.tensor_add(
                        pp[:, a:z], ppt[:, a:z], xv[:, a + 2 : z + 2]
                    )
                    v0, h0r = v1, h1

                    for t in tile_groups[ci]:
                        emit_tile(0, t)
                        emit_tile(1, t)
    nc.finalize()
    return nc


_NC_CACHE = {}


def _get_nc():
    if "nc" not in _NC_CACHE:
        _NC_CACHE["nc"] = build_nc()
    return _NC_CACHE["nc"]


def make_in_maps(x, weight, bias):
    x = np.asarray(x, dtype=np.float32)
    weight = np.asarray(weight, dtype=np.float32)
    bias = np.asarray(bias, dtype=np.float32)

    krow = weight.sum(axis=3)  # [O, I, 3]
    kcol = weight.sum(axis=2)  # [O, I, 3]
    klhs = np.empty((CI, NSL, CO), np.float32)
    klhs[:, 0, :] = (krow[:, :, 0] - krow[:, :, 2]).T
    klhs[:, 1, :] = (krow[:, :, 1] - krow[:, :, 2]).T
    klhs[:, 2, :] = (kcol[:, :, 0] - kcol[:, :, 2]).T
    klhs[:, 3, :] = (kcol[:, :, 1] - kcol[:, :, 2]).T
    klhs[:, 4, :] = (krow[:, :, 2] + kcol[:, :, 2]).T
    klhs = klhs.astype(ml_dtypes.bfloat16)

    xp = np.zeros((B, CI, HP, WP), np.float32)
    xp[:, :, 1 : H + 1, 1 : W + 1] = x
    xp = xp.astype(ml_dtypes.bfloat16)

    bias2 = np.ascontiguousarray(bias.reshape(2, 128).T)  # [128, 2] f32

    return [
        {"xp": xp[c * BPC : (c + 1) * BPC], "klhs": klhs, "bias2": bias2}
        for c in range(NCORES)
    ]


def run(in_maps, **kwargs):
    nc = _get_nc()
    return run_bass_kernel_spmd(nc, in_maps, list(range(NCORES)), **kwargs)


def kernel(x, weight, bias):
    res = run(make_in_maps(x, weight, bias))
    out = np.concatenate([res.results[c]["y"] for c in range(NCORES)], axis=0)
    return out.astype(np.float32)
